# revision 5
# baseline (speedup 1.0000x reference)
"""EnhancedCrossAttention TRN2 kernel.

Strategy: data-parallel over batch B=2048 across 8 cores (256 rows each),
weights replicated, no collectives.

Per core (b_loc=256, two b-tiles of 128):
  q-proj / k-proj / v-proj as fp16 matmuls (fp32 PSUM accumulate) with the
  activation tiles (contraction-major, pre-transposed on host) stationary and
  the weight matrix streaming as the moving operand, output layout
  [b partitions, feature free].  pos_encoding is folded on the host into a
  per-(t, o) bias (pos @ Wk.T + bk) and added during the PSUM eviction.
  k/v round-trip through DRAM scratch in fp16 (layout [h, b, t, d] so the
  attention reloads are contiguous), then attention runs on DVE/ACT:
  scores via broadcast-multiply + reduce, softmax with fused exp+row-sum on
  the scalar engine, AV via broadcast-multiply + strided reduce.  v-proj is
  split into two head-halves so the attention tail overlaps the projections.
  Attention output is PE-transposed and fed to the o-projection.
"""

import numpy as np

import concourse.bass as bass
import concourse.mybir as mybir
import concourse.tile as tile
from concourse import bacc
from concourse.bass_utils import run_bass_kernel_spmd
from concourse.masks import make_identity

B, T, D = 2048, 32, 2048
H, HD = 16, 128
NCORES = 8
BL = B // NCORES  # 256 batch rows per core

FP16 = mybir.dt.float16
FP32 = mybir.dt.float32

ITILES = D // 128   # 16 contraction tiles
OCH = D // 512      # 4 output chunks of 512 (one PSUM bank each)
INV_SQRT_HD = 1.0 / float(np.sqrt(HD))


def build_nc(b_loc=BL, nreps=1):
    nbt = b_loc // 128
    nc = bacc.Bacc("TRN2", target_bir_lowering=False, debug=False)

    queryT = nc.dram_tensor("queryT", [D, b_loc], FP16, kind="ExternalInput")
    keysT = nc.dram_tensor("keysT", [T, D, b_loc], FP16, kind="ExternalInput")
    valuesT = nc.dram_tensor("valuesT", [T, D, b_loc], FP16, kind="ExternalInput")
    wqT = nc.dram_tensor("wqT", [D, D], FP16, kind="ExternalInput")
    wkT = nc.dram_tensor("wkT", [D, D], FP16, kind="ExternalInput")
    wvT = nc.dram_tensor("wvT", [D, D], FP16, kind="ExternalInput")
    woT = nc.dram_tensor("woT", [D, D], FP16, kind="ExternalInput")
    pos_bias = nc.dram_tensor("pos_bias", [T, D], FP16, kind="ExternalInput")
    out = nc.dram_tensor("out", [b_loc, D], FP32, kind="ExternalOutput")

    X = mybir.AxisListType.X

    with tile.TileContext(nc) as tc:
        with (
            tc.tile_pool(name="consts", bufs=1) as consts,
            tc.tile_pool(name="wpool", bufs=1) as wpool,
            tc.tile_pool(name="wqs", bufs=4) as wqs_pool,
            tc.tile_pool(name="iopool", bufs=1) as iopool,
            tc.tile_pool(name="lhst", bufs=3) as lhst_pool,
            tc.tile_pool(name="evict", bufs=3) as evict_pool,
            tc.tile_pool(name="posb", bufs=2) as pos_pool,
            tc.tile_pool(name="kvpool", bufs=2) as kv_pool,
            tc.tile_pool(name="prod", bufs=1) as prod_pool,
            tc.tile_pool(name="small", bufs=4) as small_pool,
            tc.tile_pool(name="aot", bufs=2) as aot_pool,
            tc.tile_pool(name="dram", bufs=1, space="DRAM") as dram_pool,
        ):
            ident = consts.tile([128, 128], FP16)
            make_identity(nc, ident)

            for rep in range(nreps):
                # long-lived activations
                qT_sb = iopool.tile([128, ITILES, b_loc], FP16, tag="qT",
                                    name="qT_sb")
                nc.sync.dma_start(
                    out=qT_sb,
                    in_=queryT.ap().rearrange("(a p) b -> p a b", p=128),
                )
                q_sb = iopool.tile([128, nbt, D], FP16, tag="q", name="q_sb")
                p_all = [
                    iopool.tile([128, H, T], FP16, tag=f"p{bt}",
                                name=f"p_all{bt}")
                    for bt in range(nbt)
                ]
                rs_all = [
                    iopool.tile([128, H], FP32, tag=f"rs{bt}",
                                name=f"rs_all{bt}")
                    for bt in range(nbt)
                ]
                attnout = [
                    iopool.tile([128, D], FP16, tag=f"ao{bt}",
                                name=f"attnout{bt}")
                    for bt in range(nbt)
                ]

                # DRAM scratch, layout [h, b, t, d] so per-head attention
                # reloads are contiguous (t, d) runs per partition
                k_scr = [
                    dram_pool.tile([H, 128, T, HD], FP16, tag=f"kscr{bt}",
                                   name=f"k_scr{bt}")
                    for bt in range(nbt)
                ]
                v_scr = [
                    dram_pool.tile([H, 128, T, HD], FP16, tag=f"vscr{bt}",
                                   name=f"v_scr{bt}")
                    for bt in range(nbt)
                ]

                def load_weight(w_dram):
                    w_sb = wpool.tile([128, ITILES, D], FP16, tag="w",
                                      name="w_sb")
                    nc.sync.dma_start(
                        out=w_sb,
                        in_=w_dram.ap().rearrange("(a p) o -> p a o", p=128),
                    )
                    return w_sb

                def load_lhsT(src_2d, bt):
                    lt = lhst_pool.tile([128, ITILES, 128], FP16, tag="lhsT",
                                        name="lt")
                    nc.sync.dma_start(
                        out=lt,
                        in_=src_2d.rearrange("(a p) b -> p a b", p=128)[
                            :, :, bt * 128:(bt + 1) * 128
                        ],
                    )
                    return lt

                def attention_head(bt, h):
                    """AV for one head: attnout[bt][:, h] from p_all/rs_all."""
                    V_sb = kv_pool.tile([128, T, HD], FP16, tag="V",
                                        name="V_sb")
                    nc.sync.dma_start(out=V_sb, in_=v_scr[bt][h])
                    vprod = prod_pool.tile([128, T, HD], FP16, tag="prod",
                                           name="vprod")
                    p_bc = (
                        p_all[bt][:, h, :]
                        .unsqueeze(2)
                        .broadcast_to((128, T, HD))
                    )
                    nc.vector.tensor_tensor(
                        out=vprod, in0=p_bc, in1=V_sb,
                        op=mybir.AluOpType.mult,
                    )
                    acc = small_pool.tile([128, HD], FP32, tag="acc",
                                          name="acc")
                    nc.vector.tensor_reduce(
                        out=acc,
                        in_=vprod.transpose([0, 2, 1]),
                        axis=X,
                        op=mybir.AluOpType.add,
                    )
                    nc.vector.tensor_scalar_mul(
                        attnout[bt][:, h * HD:(h + 1) * HD],
                        acc,
                        rs_all[bt][:, h:h + 1],
                    )

                with tc.tile_pool(name="psA", bufs=2, space="PSUM") as psA:
                    # ---- q-projection (weights streamed in chunks so the
                    # big slot is free for wk prefetch) ----
                    pq = [
                        psA.tile([128, D], FP32, tag="pk", name=f"pq{bt}")
                        for bt in range(nbt)
                    ]
                    for it in range(ITILES):
                        for oc in range(OCH):
                            wq_c = wqs_pool.tile([128, 512], FP16, tag="wqc",
                                                 name="wq_c")
                            nc.sync.dma_start(
                                out=wq_c,
                                in_=wqT.ap()[
                                    it * 128:(it + 1) * 128,
                                    oc * 512:(oc + 1) * 512,
                                ],
                            )
                            for bt in range(nbt):
                                nc.tensor.matmul(
                                    pq[bt][:, oc * 512:(oc + 1) * 512],
                                    qT_sb[:, it, bt * 128:(bt + 1) * 128],
                                    wq_c,
                                    start=(it == 0),
                                    stop=(it == ITILES - 1),
                                )
                    for bt in range(nbt):
                        nc.scalar.copy(q_sb[:, bt, :], pq[bt])

                    # ---- k-projection (+pos bias) ----
                    wk_sb = load_weight(wkT)
                    for bt in range(nbt):
                        for t in range(T):
                            lt = load_lhsT(keysT.ap()[t], bt)
                            pos_bc = pos_pool.tile([128, D], FP16, tag="pos",
                                                   name="pos_bc")
                            nc.sync.dma_start(
                                out=pos_bc,
                                in_=pos_bias.ap()[t:t + 1, :].broadcast_to(
                                    (128, D)
                                ),
                            )
                            pk = psA.tile([128, D], FP32, tag="pk", name="pk")
                            for it in range(ITILES):
                                for oc in range(OCH):
                                    nc.tensor.matmul(
                                        pk[:, oc * 512:(oc + 1) * 512],
                                        lt[:, it, :],
                                        wk_sb[:, it, oc * 512:(oc + 1) * 512],
                                        start=(it == 0),
                                        stop=(it == ITILES - 1),
                                    )
                            k_sb = evict_pool.tile([128, D], FP16, tag="ev",
                                                   name="k_sb")
                            nc.vector.tensor_tensor(
                                out=k_sb, in0=pk, in1=pos_bc,
                                op=mybir.AluOpType.add,
                            )
                            nc.sync.dma_start(
                                out=k_scr[bt][:, :, t, :].transpose([1, 0, 2]),
                                in_=k_sb,
                            )

                    # ---- scores + softmax (needs k only) ----
                    for bt in range(nbt):
                        for h in range(H):
                            K_sb = kv_pool.tile([128, T, HD], FP16, tag="K",
                                                name="K_sb")
                            nc.sync.dma_start(out=K_sb, in_=k_scr[bt][h])
                            sprod = prod_pool.tile([128, T, HD], FP16,
                                                   tag="prod", name="sprod")
                            q_bc = (
                                q_sb[:, bt, h * HD:(h + 1) * HD]
                                .unsqueeze(1)
                                .broadcast_to((128, T, HD))
                            )
                            nc.vector.tensor_tensor(
                                out=sprod, in0=q_bc, in1=K_sb,
                                op=mybir.AluOpType.mult,
                            )
                            sraw = small_pool.tile([128, T], FP32, tag="sraw",
                                                   name="sraw")
                            nc.vector.tensor_reduce(
                                out=sraw, in_=sprod, axis=X,
                                op=mybir.AluOpType.add,
                            )
                            smax = small_pool.tile([128, 1], FP32, tag="smax",
                                                   name="smax")
                            nc.vector.tensor_reduce(
                                out=smax, in_=sraw, axis=X,
                                op=mybir.AluOpType.max,
                            )
                            negmax = small_pool.tile([128, 1], FP32,
                                                     tag="negmax",
                                                     name="negmax")
                            nc.vector.tensor_scalar_mul(
                                negmax, smax, -INV_SQRT_HD
                            )
                            se = small_pool.tile([128, 1], FP32, tag="se",
                                                 name="se")
                            nc.scalar.activation(
                                p_all[bt][:, h, :],
                                sraw,
                                mybir.ActivationFunctionType.Exp,
                                bias=negmax,
                                scale=INV_SQRT_HD,
                                accum_out=se,
                            )
                            nc.vector.reciprocal(rs_all[bt][:, h:h + 1], se)

                    # ---- v-projection in two head-halves, AV per half ----
                    wv_sb = load_weight(wvT)
                    for bt in range(nbt):
                        for oh in range(2):
                            for t in range(T):
                                lt = load_lhsT(valuesT.ap()[t], bt)
                                pv = psA.tile([128, D // 2], FP32, tag="pk",
                                              name="pv")
                                for it in range(ITILES):
                                    for oc in range(2):
                                        occ = oh * 2 + oc
                                        nc.tensor.matmul(
                                            pv[:, oc * 512:(oc + 1) * 512],
                                            lt[:, it, :],
                                            wv_sb[:, it,
                                                  occ * 512:(occ + 1) * 512],
                                            start=(it == 0),
                                            stop=(it == ITILES - 1),
                                        )
                                v_sb = evict_pool.tile([128, D // 2], FP16,
                                                       tag="ev", name="v_sb")
                                nc.scalar.copy(v_sb, pv)
                                nc.sync.dma_start(
                                    out=v_scr[bt][
                                        oh * 8:(oh + 1) * 8, :, t, :
                                    ].transpose([1, 0, 2]),
                                    in_=v_sb,
                                )
                            for h in range(oh * 8, (oh + 1) * 8):
                                attention_head(bt, h)

                # psA closed; transposes + o-projection use fresh PSUM space
                wo_sb = load_weight(woT)
                with tc.tile_pool(name="psB", bufs=2, space="PSUM") as psB:
                    for bt in range(nbt):
                        aoT = aot_pool.tile([128, ITILES, 128], FP16,
                                            tag="aoT", name="aoT")
                        for it in range(ITILES):
                            pt = psB.tile([128, 128], FP16, tag="pt",
                                          name="pt")
                            nc.tensor.transpose(
                                pt, attnout[bt][:, it * 128:(it + 1) * 128],
                                ident,
                            )
                            nc.scalar.copy(aoT[:, it, :], pt)
                        po = psB.tile([128, D], FP32, tag="po", bufs=1,
                                      name="po")
                        for it in range(ITILES):
                            for oc in range(OCH):
                                nc.tensor.matmul(
                                    po[:, oc * 512:(oc + 1) * 512],
                                    aoT[:, it, :],
                                    wo_sb[:, it, oc * 512:(oc + 1) * 512],
                                    start=(it == 0),
                                    stop=(it == ITILES - 1),
                                )
                        out_sb = evict_pool.tile([128, D], FP32, tag="osb",
                                                 bufs=2, name=f"out_sb{bt}")
                        nc.scalar.copy(out_sb, po)
                        nc.sync.dma_start(
                            out=out.ap()[bt * 128:(bt + 1) * 128, :],
                            in_=out_sb,
                        )

    nc.compile()
    return nc


def host_prep(query, keys, values, mask, pos_encoding, Wq, bq, Wk, bk, Wv, bv,
              Wo, bo):
    """Build per-core input maps.  All heavy tensors pre-transposed to
    contraction-major layout and cast to fp16 on the host."""
    query = np.asarray(query, dtype=np.float32)
    keys = np.asarray(keys, dtype=np.float32)
    values = np.asarray(values, dtype=np.float32)
    pos_encoding = np.asarray(pos_encoding, dtype=np.float32)
    Wq, Wk, Wv, Wo = (np.asarray(w, dtype=np.float32) for w in (Wq, Wk, Wv, Wo))
    bk = np.asarray(bk, dtype=np.float32)

    wqT = np.ascontiguousarray(Wq.T).astype(np.float16)
    wkT = np.ascontiguousarray(Wk.T).astype(np.float16)
    wvT = np.ascontiguousarray(Wv.T).astype(np.float16)
    woT = np.ascontiguousarray(Wo.T).astype(np.float16)

    pos = np.clip(pos_encoding[:T], -10.0, 10.0)
    pos_bias = (pos @ Wk.T + bk).astype(np.float16)  # (T, D)

    in_maps = []
    for c in range(NCORES):
        sl = slice(c * BL, (c + 1) * BL)
        in_maps.append({
            "queryT": np.ascontiguousarray(query[sl].T).astype(np.float16),
            "keysT": np.ascontiguousarray(
                keys[:, sl, :].transpose(0, 2, 1)).astype(np.float16),
            "valuesT": np.ascontiguousarray(
                values[:, sl, :].transpose(0, 2, 1)).astype(np.float16),
            "wqT": wqT, "wkT": wkT, "wvT": wvT, "woT": woT,
            "pos_bias": pos_bias,
        })
    return in_maps


_STATE = {}


def _get_nc():
    if "nc" not in _STATE:
        _STATE["nc"] = build_nc()
    return _STATE["nc"]


def run_on_hw(in_maps, trace=False):
    nc = _get_nc()
    return run_bass_kernel_spmd(nc, in_maps, list(range(NCORES)), trace=trace)


def kernel(**inputs):
    in_maps = host_prep(**inputs)
    res = run_on_hw(in_maps)
    return np.concatenate(
        [np.asarray(res.results[c]["out"]) for c in range(NCORES)], axis=0
    )


# revision 13
# speedup vs baseline: 1.2426x; 1.2426x over previous
"""EnhancedCrossAttention TRN2 kernel.

Strategy: data-parallel over batch B=2048 across 8 cores (256 rows each),
weights replicated, no collectives.

Per core (b_loc=256, two b-tiles of 128):
  q-proj / k-proj / v-proj as fp16 matmuls (fp32 PSUM accumulate) with the
  activation tiles (contraction-major, pre-transposed on host) stationary and
  the weight matrix streaming as the moving operand, output layout
  [b partitions, feature free].  pos_encoding is folded on the host into a
  per-(t, o) bias (pos @ Wk.T + bk) and added during the PSUM eviction.
  k/v round-trip through DRAM scratch in fp16 (layout [h, b, t, d] so the
  attention reloads are contiguous), then attention runs on DVE/ACT:
  scores via broadcast-multiply + reduce, softmax with fused exp+row-sum on
  the scalar engine, AV via broadcast-multiply + strided reduce.  v-proj is
  split into two head-halves so the attention tail overlaps the projections.
  Attention output is PE-transposed and fed to the o-projection.
"""

import numpy as np

import concourse.bass as bass
import concourse.mybir as mybir
import concourse.tile as tile
from concourse import bacc
from concourse.bass_utils import run_bass_kernel_spmd
from concourse.masks import make_identity

B, T, D = 2048, 32, 2048
H, HD = 16, 128
NCORES = 8
BL = B // NCORES  # 256 batch rows per core

FP16 = mybir.dt.float16
FP32 = mybir.dt.float32

ITILES = D // 128   # 16 contraction tiles
OCH = D // 512      # 4 output chunks of 512 (one PSUM bank each)
INV_SQRT_HD = 1.0 / float(np.sqrt(HD))


def build_nc(b_loc=BL, nreps=1):
    nbt = b_loc // 128
    nc = bacc.Bacc("TRN2", target_bir_lowering=False, debug=False)

    queryT = nc.dram_tensor("queryT", [D, b_loc], FP16, kind="ExternalInput")
    keysT = nc.dram_tensor("keysT", [T, D, b_loc], FP16, kind="ExternalInput")
    valuesT = nc.dram_tensor("valuesT", [T, D, b_loc], FP16, kind="ExternalInput")
    wqT = nc.dram_tensor("wqT", [D, D], FP16, kind="ExternalInput")
    wkT = nc.dram_tensor("wkT", [D, D], FP16, kind="ExternalInput")
    wvT = nc.dram_tensor("wvT", [D, D], FP16, kind="ExternalInput")
    woT = nc.dram_tensor("woT", [D, D], FP16, kind="ExternalInput")
    pos_bias = nc.dram_tensor("pos_bias", [T, D], FP16, kind="ExternalInput")
    out = nc.dram_tensor("out", [b_loc, D], FP32, kind="ExternalOutput")

    X = mybir.AxisListType.X

    with tile.TileContext(nc) as tc:
        with (
            tc.tile_pool(name="consts", bufs=1) as consts,
            tc.tile_pool(name="wpool", bufs=1) as wpool,
            tc.tile_pool(name="wqs", bufs=4) as wqs_pool,
            tc.tile_pool(name="iopool", bufs=1) as iopool,
            tc.tile_pool(name="lhst", bufs=3) as lhst_pool,
            tc.tile_pool(name="evict", bufs=3) as evict_pool,
            tc.tile_pool(name="posb", bufs=2) as pos_pool,
            tc.tile_pool(name="kvpool", bufs=2) as kv_pool,
            tc.tile_pool(name="prod", bufs=1) as prod_pool,
            tc.tile_pool(name="small", bufs=4) as small_pool,
            tc.tile_pool(name="aot", bufs=2) as aot_pool,
            tc.tile_pool(name="dram", bufs=1, space="DRAM") as dram_pool,
        ):
            ident = consts.tile([128, 128], FP16)
            make_identity(nc, ident)

            for rep in range(nreps):
                # long-lived activations
                qT_sb = iopool.tile([128, ITILES, b_loc], FP16, tag="qT",
                                    name="qT_sb")
                nc.sync.dma_start(
                    out=qT_sb,
                    in_=queryT.ap().rearrange("(a p) b -> p a b", p=128),
                )
                q_sb = iopool.tile([128, nbt, D], FP16, tag="q", name="q_sb")
                p_all = [
                    iopool.tile([128, H, T], FP16, tag=f"p{bt}",
                                name=f"p_all{bt}")
                    for bt in range(nbt)
                ]
                rs_all = [
                    iopool.tile([128, H], FP32, tag=f"rs{bt}",
                                name=f"rs_all{bt}")
                    for bt in range(nbt)
                ]
                attnout = [
                    iopool.tile([128, D], FP16, tag=f"ao{bt}",
                                name=f"attnout{bt}")
                    for bt in range(nbt)
                ]

                # DRAM scratch, layout [h, b, t, d] so per-head attention
                # reloads are contiguous (t, d) runs per partition
                k_scr = [
                    dram_pool.tile([H, 128, T, HD], FP16, tag=f"kscr{bt}",
                                   name=f"k_scr{bt}")
                    for bt in range(nbt)
                ]
                # asymmetric split (12 heads then 4): the trailing AV
                # block after the last v-proj chunk covers only 4 heads.
                # Separate tiles per chunk avoid WAR serialization between
                # chunk 1's writes and chunk 0's attention reads.
                VSPLIT = (8, 8)
                v_scr = [
                    [
                        dram_pool.tile([nh, 128, T, HD], FP16,
                                       tag=f"vscr{bt}_{oh}",
                                       name=f"v_scr{bt}_{oh}")
                        for oh, nh in enumerate(VSPLIT)
                    ]
                    for bt in range(nbt)
                ]

                def load_weight(w_dram):
                    w_sb = wpool.tile([128, ITILES, D], FP16, tag="w",
                                      name="w_sb")
                    nc.sync.dma_start(
                        out=w_sb,
                        in_=w_dram.ap().rearrange("(a p) o -> p a o", p=128),
                    )
                    return w_sb

                def load_lhsT(src_2d, bt):
                    lt = lhst_pool.tile([128, ITILES, 128], FP16, tag="lhsT",
                                        name="lt")
                    nc.sync.dma_start(
                        out=lt,
                        in_=src_2d.rearrange("(a p) b -> p a b", p=128)[
                            :, :, bt * 128:(bt + 1) * 128
                        ],
                    )
                    return lt

                def attention_head(bt, h):
                    """AV for one head: attnout[bt][:, h] from p_all/rs_all."""
                    V_sb = kv_pool.tile([128, T, HD], FP16, tag="V",
                                        name="V_sb")
                    nc.sync.dma_start(
                        out=V_sb,
                        in_=(v_scr[bt][0][h] if h < 8
                             else v_scr[bt][1][h - 8]),
                    )
                    vprod = prod_pool.tile([128, T, HD], FP16, tag="prod",
                                           name="vprod")
                    p_bc = (
                        p_all[bt][:, h, :]
                        .unsqueeze(2)
                        .broadcast_to((128, T, HD))
                    )
                    nc.vector.tensor_tensor(
                        out=vprod, in0=p_bc, in1=V_sb,
                        op=mybir.AluOpType.mult,
                    )
                    acc = small_pool.tile([128, HD], FP32, tag="acc",
                                          name="acc")
                    nc.vector.tensor_reduce(
                        out=acc,
                        in_=vprod.transpose([0, 2, 1]),
                        axis=X,
                        op=mybir.AluOpType.add,
                    )
                    nc.vector.tensor_scalar_mul(
                        attnout[bt][:, h * HD:(h + 1) * HD],
                        acc,
                        rs_all[bt][:, h:h + 1],
                    )

                with tc.tile_pool(name="psA", bufs=2, space="PSUM") as psA:
                    # ---- q-projection (weights streamed in chunks so the
                    # big slot is free for wk prefetch) ----
                    pq = [
                        psA.tile([128, D], FP32, tag="pk", name=f"pq{bt}")
                        for bt in range(nbt)
                    ]
                    for it in range(ITILES):
                        for oc in range(OCH):
                            wq_c = wqs_pool.tile([128, 512], FP16, tag="wqc",
                                                 name="wq_c")
                            nc.sync.dma_start(
                                out=wq_c,
                                in_=wqT.ap()[
                                    it * 128:(it + 1) * 128,
                                    oc * 512:(oc + 1) * 512,
                                ],
                            )
                            for bt in range(nbt):
                                nc.tensor.matmul(
                                    pq[bt][:, oc * 512:(oc + 1) * 512],
                                    qT_sb[:, it, bt * 128:(bt + 1) * 128],
                                    wq_c,
                                    start=(it == 0),
                                    stop=(it == ITILES - 1),
                                )
                    for bt in range(nbt):
                        nc.scalar.copy(q_sb[:, bt, :], pq[bt])

                    # ---- k-projection (+pos bias) ----
                    wk_sb = load_weight(wkT)
                    for bt in range(nbt):
                        for t in range(T):
                            lt = load_lhsT(keysT.ap()[t], bt)
                            pos_bc = pos_pool.tile([128, D], FP16, tag="pos",
                                                   name="pos_bc")
                            nc.sync.dma_start(
                                out=pos_bc,
                                in_=pos_bias.ap()[t:t + 1, :].broadcast_to(
                                    (128, D)
                                ),
                            )
                            pk = psA.tile([128, D], FP32, tag="pk", name="pk")
                            for it in range(ITILES):
                                for oc in range(OCH):
                                    nc.tensor.matmul(
                                        pk[:, oc * 512:(oc + 1) * 512],
                                        lt[:, it, :],
                                        wk_sb[:, it, oc * 512:(oc + 1) * 512],
                                        start=(it == 0),
                                        stop=(it == ITILES - 1),
                                    )
                            k_sb = evict_pool.tile([128, D], FP16, tag="ev",
                                                   name="k_sb")
                            nc.vector.tensor_tensor(
                                out=k_sb, in0=pk, in1=pos_bc,
                                op=mybir.AluOpType.add,
                            )
                            nc.sync.dma_start(
                                out=k_scr[bt][:, :, t, :].transpose([1, 0, 2]),
                                in_=k_sb,
                            )

                    # ---- scores + softmax (needs k only) ----
                    for bt in range(nbt):
                        for h in range(H):
                            K_sb = kv_pool.tile([128, T, HD], FP16, tag="K",
                                                name="K_sb")
                            nc.sync.dma_start(out=K_sb, in_=k_scr[bt][h])
                            sprod = prod_pool.tile([128, T, HD], FP16,
                                                   tag="prod", name="sprod")
                            q_bc = (
                                q_sb[:, bt, h * HD:(h + 1) * HD]
                                .unsqueeze(1)
                                .broadcast_to((128, T, HD))
                            )
                            nc.vector.tensor_tensor(
                                out=sprod, in0=q_bc, in1=K_sb,
                                op=mybir.AluOpType.mult,
                            )
                            sraw = small_pool.tile([128, T], FP32, tag="sraw",
                                                   name="sraw")
                            nc.vector.tensor_reduce(
                                out=sraw, in_=sprod, axis=X,
                                op=mybir.AluOpType.add,
                            )
                            smax = small_pool.tile([128, 1], FP32, tag="smax",
                                                   name="smax")
                            nc.vector.tensor_reduce(
                                out=smax, in_=sraw, axis=X,
                                op=mybir.AluOpType.max,
                            )
                            negmax = small_pool.tile([128, 1], FP32,
                                                     tag="negmax",
                                                     name="negmax")
                            nc.vector.tensor_scalar_mul(
                                negmax, smax, -INV_SQRT_HD
                            )
                            se = small_pool.tile([128, 1], FP32, tag="se",
                                                 name="se")
                            nc.scalar.activation(
                                p_all[bt][:, h, :],
                                sraw,
                                mybir.ActivationFunctionType.Exp,
                                bias=negmax,
                                scale=INV_SQRT_HD,
                                accum_out=se,
                            )
                            nc.vector.reciprocal(rs_all[bt][:, h:h + 1], se)

                # psQK closed; v-proj + o-proj use separate PSUM pools so
                # o-proj(bt0) can run while v-proj(bt1) is still active
                wv_sb = load_weight(wvT)
                wo_sb = None
                with (
                    tc.tile_pool(name="psV", bufs=2, space="PSUM") as psV,
                    tc.tile_pool(name="psB", bufs=1, space="PSUM") as psB,
                ):
                    for bt in range(nbt):
                        # v-projection in asymmetric head chunks (12 then 4),
                        # AV per chunk: overlaps attention with projections
                        # and leaves only a 4-head trailing block
                        h0 = 0
                        for oh, nh in enumerate(VSPLIT):
                            noc = nh // 4  # 512-wide chunks
                            for t in range(T):
                                lt = load_lhsT(valuesT.ap()[t], bt)
                                pv = psV.tile([128, nh * HD], FP32, tag="pv",
                                              name="pv",
                                              padded_shape=[128, 1024])
                                for it in range(ITILES):
                                    for oc in range(noc):
                                        occ = h0 // 4 + oc
                                        nc.tensor.matmul(
                                            pv[:, oc * 512:(oc + 1) * 512],
                                            lt[:, it, :],
                                            wv_sb[:, it,
                                                  occ * 512:(occ + 1) * 512],
                                            start=(it == 0),
                                            stop=(it == ITILES - 1),
                                        )
                                v_sb = evict_pool.tile([128, nh * HD], FP16,
                                                       tag="ev", name="v_sb")
                                nc.scalar.copy(v_sb, pv)
                                nc.sync.dma_start(
                                    out=v_scr[bt][oh][:, :, t, :].transpose(
                                        [1, 0, 2]
                                    ),
                                    in_=v_sb,
                                )
                            for h in range(h0, h0 + nh):
                                attention_head(bt, h)
                            h0 += nh
                        if bt == 0:
                            wo_sb = load_weight(woT)

                        # o-projection for this b-tile
                        aoT = aot_pool.tile([128, ITILES, 128], FP16,
                                            tag="aoT", name="aoT")
                        for it in range(ITILES):
                            pt = psB.tile([128, 128], FP16, tag="pt",
                                          bufs=2, name="pt")
                            nc.tensor.transpose(
                                pt, attnout[bt][:, it * 128:(it + 1) * 128],
                                ident,
                            )
                            nc.scalar.copy(aoT[:, it, :], pt)
                        for half in range(2):
                            po = psB.tile([128, D // 2], FP32, tag="po",
                                          bufs=1, name="po")
                            for it in range(ITILES):
                                for oc in range(2):
                                    occ = half * 2 + oc
                                    nc.tensor.matmul(
                                        po[:, oc * 512:(oc + 1) * 512],
                                        aoT[:, it, :],
                                        wo_sb[:, it,
                                              occ * 512:(occ + 1) * 512],
                                        start=(it == 0),
                                        stop=(it == ITILES - 1),
                                    )
                            out_sb = evict_pool.tile(
                                [128, D // 2], FP32, tag="osb", bufs=2,
                                name="out_sb"
                            )
                            nc.scalar.copy(out_sb, po)
                            nc.sync.dma_start(
                                out=out.ap()[
                                    bt * 128:(bt + 1) * 128,
                                    half * 1024:(half + 1) * 1024,
                                ],
                                in_=out_sb,
                            )

    nc.compile()
    return nc


def host_prep(query, keys, values, mask, pos_encoding, Wq, bq, Wk, bk, Wv, bv,
              Wo, bo):
    """Build per-core input maps.  All heavy tensors pre-transposed to
    contraction-major layout and cast to fp16 on the host."""
    query = np.asarray(query, dtype=np.float32)
    keys = np.asarray(keys, dtype=np.float32)
    values = np.asarray(values, dtype=np.float32)
    pos_encoding = np.asarray(pos_encoding, dtype=np.float32)
    Wq, Wk, Wv, Wo = (np.asarray(w, dtype=np.float32) for w in (Wq, Wk, Wv, Wo))
    bk = np.asarray(bk, dtype=np.float32)

    wqT = np.ascontiguousarray(Wq.T).astype(np.float16)
    wkT = np.ascontiguousarray(Wk.T).astype(np.float16)
    wvT = np.ascontiguousarray(Wv.T).astype(np.float16)
    woT = np.ascontiguousarray(Wo.T).astype(np.float16)

    pos = np.clip(pos_encoding[:T], -10.0, 10.0)
    pos_bias = (pos @ Wk.T + bk).astype(np.float16)  # (T, D)

    in_maps = []
    for c in range(NCORES):
        sl = slice(c * BL, (c + 1) * BL)
        in_maps.append({
            "queryT": np.ascontiguousarray(query[sl].T).astype(np.float16),
            "keysT": np.ascontiguousarray(
                keys[:, sl, :].transpose(0, 2, 1)).astype(np.float16),
            "valuesT": np.ascontiguousarray(
                values[:, sl, :].transpose(0, 2, 1)).astype(np.float16),
            "wqT": wqT, "wkT": wkT, "wvT": wvT, "woT": woT,
            "pos_bias": pos_bias,
        })
    return in_maps


_STATE = {}


def _get_nc():
    if "nc" not in _STATE:
        _STATE["nc"] = build_nc()
    return _STATE["nc"]


def run_on_hw(in_maps, trace=False):
    nc = _get_nc()
    return run_bass_kernel_spmd(nc, in_maps, list(range(NCORES)), trace=trace)


def kernel(**inputs):
    in_maps = host_prep(**inputs)
    res = run_on_hw(in_maps)
    return np.concatenate(
        [np.asarray(res.results[c]["out"]) for c in range(NCORES)], axis=0
    )


# revision 15
# speedup vs baseline: 1.4379x; 1.1572x over previous
"""EnhancedCrossAttention TRN2 kernel.

Strategy: data-parallel over batch B=2048 across 8 cores (256 rows each),
weights replicated, no collectives.

Per core (b_loc=256, two b-tiles of 128):
  q-proj / k-proj / v-proj as fp16 matmuls (fp32 PSUM accumulate) with the
  activation tiles (contraction-major, pre-transposed on host) stationary and
  the weight matrix streaming as the moving operand, output layout
  [b partitions, feature free].  pos_encoding is folded on the host into a
  per-(t, o) bias (pos @ Wk.T + bk) and added during the PSUM eviction.
  k/v round-trip through DRAM scratch in fp16 (layout [h, b, t, d] so the
  attention reloads are contiguous), then attention runs on DVE/ACT:
  scores via broadcast-multiply + reduce, softmax with fused exp+row-sum on
  the scalar engine, AV via broadcast-multiply + strided reduce.  v-proj is
  split into two head-halves so the attention tail overlaps the projections.
  Attention output is PE-transposed and fed to the o-projection.
"""

import numpy as np

import concourse.bass as bass
import concourse.mybir as mybir
import concourse.tile as tile
from concourse import bacc
from concourse.bass_utils import run_bass_kernel_spmd
from concourse.masks import make_identity

B, T, D = 2048, 32, 2048
H, HD = 16, 128
NCORES = 8
BL = B // NCORES  # 256 batch rows per core

FP16 = mybir.dt.float16
FP32 = mybir.dt.float32

ITILES = D // 128   # 16 contraction tiles
OCH = D // 512      # 4 output chunks of 512 (one PSUM bank each)
INV_SQRT_HD = 1.0 / float(np.sqrt(HD))


def build_nc(b_loc=BL, nreps=1):
    nbt = b_loc // 128
    nc = bacc.Bacc("TRN2", target_bir_lowering=False, debug=False)

    queryT = nc.dram_tensor("queryT", [D, b_loc], FP16, kind="ExternalInput")
    keysT = nc.dram_tensor("keysT", [T, D, b_loc], FP16, kind="ExternalInput")
    valuesT = nc.dram_tensor("valuesT", [T, D, b_loc], FP16, kind="ExternalInput")
    wqT = nc.dram_tensor("wqT", [D, D], FP16, kind="ExternalInput")
    wkT = nc.dram_tensor("wkT", [D, D], FP16, kind="ExternalInput")
    wvT = nc.dram_tensor("wvT", [D, D], FP16, kind="ExternalInput")
    woT = nc.dram_tensor("woT", [D, D], FP16, kind="ExternalInput")
    pos_bias = nc.dram_tensor("pos_bias", [T, D], FP16, kind="ExternalInput")
    out = nc.dram_tensor("out", [b_loc, D], FP32, kind="ExternalOutput")

    X = mybir.AxisListType.X
    MULT = mybir.AluOpType.mult
    ADD = mybir.AluOpType.add

    with tile.TileContext(nc) as tc:
        with (
            tc.tile_pool(name="consts", bufs=1) as consts,
            tc.tile_pool(name="wpool", bufs=1) as wpool,
            tc.tile_pool(name="wqs", bufs=8) as wqs_pool,
            tc.tile_pool(name="iopool", bufs=1) as iopool,
            tc.tile_pool(name="lhst", bufs=3) as lhst_pool,
            tc.tile_pool(name="evict", bufs=3) as evict_pool,
            tc.tile_pool(name="posb", bufs=2) as pos_pool,
            tc.tile_pool(name="prod", bufs=2) as prod_pool,
            tc.tile_pool(name="small", bufs=4) as small_pool,
            tc.tile_pool(name="aot", bufs=2) as aot_pool,
        ):
            ident = consts.tile([128, 128], FP16)
            make_identity(nc, ident)

            for rep in range(nreps):
                qT_sb = iopool.tile([128, ITILES, b_loc], FP16, tag="qT",
                                    name="qT_sb")
                nc.sync.dma_start(
                    out=qT_sb,
                    in_=queryT.ap().rearrange("(a p) b -> p a b", p=128),
                )
                q_sb = iopool.tile([128, nbt, D], FP16, tag="q", name="q_sb")
                # raw scores [b, t, h], filled one t-slice per k row-tile
                sc = [
                    iopool.tile([128, T, H], FP32, tag=f"sc{bt}",
                                name=f"sc{bt}")
                    for bt in range(nbt)
                ]
                p_all = [
                    iopool.tile([128, H, T], FP32, tag=f"p{bt}",
                                name=f"p_all{bt}")
                    for bt in range(nbt)
                ]
                rs_all = [
                    iopool.tile([128, H], FP32, tag=f"rs{bt}",
                                name=f"rs_all{bt}")
                    for bt in range(nbt)
                ]
                # fp32 AV accumulators (one per b-tile), normalized+cast at
                # the end into attnout16
                acc = [
                    iopool.tile([128, D], FP32, tag=f"acc{bt}",
                                name=f"acc{bt}")
                    for bt in range(nbt)
                ]
                attnout = [
                    iopool.tile([128, D], FP16, tag=f"ao{bt}",
                                name=f"attnout{bt}")
                    for bt in range(nbt)
                ]

                def load_weight(w_dram):
                    w_sb = wpool.tile([128, ITILES, D], FP16, tag="w",
                                      name="w_sb")
                    nc.sync.dma_start(
                        out=w_sb,
                        in_=w_dram.ap().rearrange("(a p) o -> p a o", p=128),
                    )
                    return w_sb

                def load_wchunk(w_dram, it, occ):
                    wc = wqs_pool.tile([128, 512], FP16, tag="wqc",
                                       name="w_c")
                    nc.sync.dma_start(
                        out=wc,
                        in_=w_dram.ap()[
                            it * 128:(it + 1) * 128, occ * 512:(occ + 1) * 512
                        ],
                    )
                    return wc

                def load_lhsT(src_2d, bt):
                    lt = lhst_pool.tile([128, ITILES, 128], FP16, tag="lhsT",
                                        name="lt")
                    nc.sync.dma_start(
                        out=lt,
                        in_=src_2d.rearrange("(a p) b -> p a b", p=128)[
                            :, :, bt * 128:(bt + 1) * 128
                        ],
                    )
                    return lt

                with tc.tile_pool(name="psA", bufs=2, space="PSUM") as psA:
                    # ---- q-projection (weights streamed in chunks so the
                    # big slot is free for wk prefetch) ----
                    pq = [
                        psA.tile([128, D], FP32, tag="pk", name=f"pq{bt}")
                        for bt in range(nbt)
                    ]
                    for it in range(ITILES):
                        for oc in range(OCH):
                            wq_c = load_wchunk(wqT, it, oc)
                            for bt in range(nbt):
                                nc.tensor.matmul(
                                    pq[bt][:, oc * 512:(oc + 1) * 512],
                                    qT_sb[:, it, bt * 128:(bt + 1) * 128],
                                    wq_c,
                                    start=(it == 0),
                                    stop=(it == ITILES - 1),
                                )
                    for bt in range(nbt):
                        nc.scalar.copy(q_sb[:, bt, :], pq[bt])

                    # ---- k-projection with inline scores: k never leaves
                    # SBUF.  Per row-tile: k = psum + pos bias, then one
                    # q*k multiply and a per-head reduce give the raw
                    # scores for all 16 heads at this t. ----
                    wk_sb = load_weight(wkT)
                    for bt in range(nbt):
                        for t in range(T):
                            lt = load_lhsT(keysT.ap()[t], bt)
                            pos_bc = pos_pool.tile([128, D], FP16, tag="pos",
                                                   name="pos_bc")
                            nc.sync.dma_start(
                                out=pos_bc,
                                in_=pos_bias.ap()[t:t + 1, :].broadcast_to(
                                    (128, D)
                                ),
                            )
                            pk = psA.tile([128, D], FP32, tag="pk", name="pk")
                            for it in range(ITILES):
                                for oc in range(OCH):
                                    nc.tensor.matmul(
                                        pk[:, oc * 512:(oc + 1) * 512],
                                        lt[:, it, :],
                                        wk_sb[:, it, oc * 512:(oc + 1) * 512],
                                        start=(it == 0),
                                        stop=(it == ITILES - 1),
                                    )
                            k_sb = evict_pool.tile([128, D], FP16, tag="ev",
                                                   name="k_sb")
                            nc.vector.tensor_tensor(
                                out=k_sb, in0=pk, in1=pos_bc, op=ADD,
                            )
                            sprod = prod_pool.tile([128, D], FP16,
                                                   tag="prod", name="sprod")
                            nc.vector.tensor_tensor(
                                out=sprod, in0=q_sb[:, bt, :], in1=k_sb,
                                op=MULT,
                            )
                            nc.vector.tensor_reduce(
                                out=sc[bt][:, t, :],
                                in_=sprod.rearrange("p (h d) -> p h d", h=H),
                                axis=X,
                                op=ADD,
                            )
                        # softmax for this b-tile (overlaps next phases)
                        for h in range(H):
                            smax = small_pool.tile([128, 1], FP32, tag="smax",
                                                   name="smax")
                            nc.vector.tensor_reduce(
                                out=smax, in_=sc[bt][:, :, h], axis=X,
                                op=mybir.AluOpType.max,
                            )
                            negmax = small_pool.tile([128, 1], FP32,
                                                     tag="negmax",
                                                     name="negmax")
                            nc.vector.tensor_scalar_mul(
                                negmax, smax, -INV_SQRT_HD
                            )
                            se = small_pool.tile([128, 1], FP32, tag="se",
                                                 name="se")
                            nc.scalar.activation(
                                p_all[bt][:, h, :],
                                sc[bt][:, :, h],
                                mybir.ActivationFunctionType.Exp,
                                bias=negmax,
                                scale=INV_SQRT_HD,
                                accum_out=se,
                            )
                            nc.vector.reciprocal(rs_all[bt][:, h:h + 1], se)

                # psA closed; v-proj + o-proj use separate PSUM pools
                wv_sb = load_weight(wvT)
                with (
                    tc.tile_pool(name="psV", bufs=2, space="PSUM") as psV,
                    tc.tile_pool(name="psB", bufs=1, space="PSUM") as psB,
                ):
                    for bt in range(nbt):
                        # v-projection in head-halves; AV accumulates
                        # directly from PSUM via fused (pv*p)+acc, so v
                        # never leaves the chip either
                        for oh in range(2):
                            for t in range(T):
                                lt = load_lhsT(valuesT.ap()[t], bt)
                                pv = psV.tile([128, D // 2], FP32, tag="pv",
                                              name="pv")
                                for it in range(ITILES):
                                    for oc in range(2):
                                        occ = oh * 2 + oc
                                        nc.tensor.matmul(
                                            pv[:, oc * 512:(oc + 1) * 512],
                                            lt[:, it, :],
                                            wv_sb[:, it,
                                                  occ * 512:(occ + 1) * 512],
                                            start=(it == 0),
                                            stop=(it == ITILES - 1),
                                        )
                                for hh in range(8):
                                    h = oh * 8 + hh
                                    hsl = slice(h * HD, (h + 1) * HD)
                                    psl = pv[:, hh * HD:(hh + 1) * HD]
                                    pcol = p_all[bt][:, h, t:t + 1]
                                    if t == 0:
                                        nc.vector.tensor_scalar_mul(
                                            acc[bt][:, hsl], psl, pcol
                                        )
                                    else:
                                        nc.vector.scalar_tensor_tensor(
                                            out=acc[bt][:, hsl],
                                            in0=psl,
                                            scalar=pcol,
                                            in1=acc[bt][:, hsl],
                                            op0=MULT,
                                            op1=ADD,
                                        )
                        # normalize by 1/sum(exp) and cast to fp16
                        for h in range(H):
                            hsl = slice(h * HD, (h + 1) * HD)
                            nc.vector.tensor_scalar_mul(
                                attnout[bt][:, hsl], acc[bt][:, hsl],
                                rs_all[bt][:, h:h + 1],
                            )
                        # o-projection for this b-tile (wo streamed in
                        # chunks: no second big weight slot needed)
                        aoT = aot_pool.tile([128, ITILES, 128], FP16,
                                            tag="aoT", name="aoT")
                        for it in range(ITILES):
                            pt = psB.tile([128, 128], FP16, tag="pt",
                                          bufs=2, name="pt")
                            nc.tensor.transpose(
                                pt, attnout[bt][:, it * 128:(it + 1) * 128],
                                ident,
                            )
                            nc.scalar.copy(aoT[:, it, :], pt)
                        for half in range(2):
                            po = psB.tile([128, D // 2], FP32, tag="po",
                                          bufs=1, name="po")
                            for it in range(ITILES):
                                for oc in range(2):
                                    occ = half * 2 + oc
                                    wo_c = load_wchunk(woT, it, occ)
                                    nc.tensor.matmul(
                                        po[:, oc * 512:(oc + 1) * 512],
                                        aoT[:, it, :],
                                        wo_c,
                                        start=(it == 0),
                                        stop=(it == ITILES - 1),
                                    )
                            out_sb = evict_pool.tile(
                                [128, D // 2], FP32, tag="osb", bufs=2,
                                name="out_sb"
                            )
                            nc.scalar.copy(out_sb, po)
                            nc.sync.dma_start(
                                out=out.ap()[
                                    bt * 128:(bt + 1) * 128,
                                    half * 1024:(half + 1) * 1024,
                                ],
                                in_=out_sb,
                            )

    nc.compile()
    return nc


def host_prep(query, keys, values, mask, pos_encoding, Wq, bq, Wk, bk, Wv, bv,
              Wo, bo):
    """Build per-core input maps.  All heavy tensors pre-transposed to
    contraction-major layout and cast to fp16 on the host."""
    query = np.asarray(query, dtype=np.float32)
    keys = np.asarray(keys, dtype=np.float32)
    values = np.asarray(values, dtype=np.float32)
    pos_encoding = np.asarray(pos_encoding, dtype=np.float32)
    Wq, Wk, Wv, Wo = (np.asarray(w, dtype=np.float32) for w in (Wq, Wk, Wv, Wo))
    bk = np.asarray(bk, dtype=np.float32)

    wqT = np.ascontiguousarray(Wq.T).astype(np.float16)
    wkT = np.ascontiguousarray(Wk.T).astype(np.float16)
    wvT = np.ascontiguousarray(Wv.T).astype(np.float16)
    woT = np.ascontiguousarray(Wo.T).astype(np.float16)

    pos = np.clip(pos_encoding[:T], -10.0, 10.0)
    pos_bias = (pos @ Wk.T + bk).astype(np.float16)  # (T, D)

    in_maps = []
    for c in range(NCORES):
        sl = slice(c * BL, (c + 1) * BL)
        in_maps.append({
            "queryT": np.ascontiguousarray(query[sl].T).astype(np.float16),
            "keysT": np.ascontiguousarray(
                keys[:, sl, :].transpose(0, 2, 1)).astype(np.float16),
            "valuesT": np.ascontiguousarray(
                values[:, sl, :].transpose(0, 2, 1)).astype(np.float16),
            "wqT": wqT, "wkT": wkT, "wvT": wvT, "woT": woT,
            "pos_bias": pos_bias,
        })
    return in_maps


_STATE = {}


def _get_nc():
    if "nc" not in _STATE:
        _STATE["nc"] = build_nc()
    return _STATE["nc"]


def run_on_hw(in_maps, trace=False):
    nc = _get_nc()
    return run_bass_kernel_spmd(nc, in_maps, list(range(NCORES)), trace=trace)


def kernel(**inputs):
    in_maps = host_prep(**inputs)
    res = run_on_hw(in_maps)
    return np.concatenate(
        [np.asarray(res.results[c]["out"]) for c in range(NCORES)], axis=0
    )
